# revision 32
# baseline (speedup 1.0000x reference)
"""DistMult decoder kernel for Trainium2 (8 NeuronCores, SPMD).

Computes rec = (inputs * relation) @ inputs.T for inputs [8192, 512] f32,
relation [512] f32, output [8192, 8192] f32.

Strategy: the output is symmetric (rec[m,n] = sum_k r_k x_mk x_nk), so only
the 136 lower-triangle 512x512 blocks (of 16x16) are computed on device; the
mirror happens during host-side assembly.  The triangle is balanced across 8
cores with a rotation trick that keeps the program SPMD-uniform: core c gets
X row-rotated by c*512, computes local row-block 0 against local col-blocks
0..8 and local row-block 8 against local col-blocks 8..15 (17 blocks each).
Matmuls run in bf16 (fp32 PSUM accumulation); scaling by relation is done on
host in fp32 before the bf16 cast.  The host pre-tiles both operands into
the exact SBUF layout so all device DMAs are large and contiguous.
"""

import numpy as np
import ml_dtypes

import concourse.bass as bass
import concourse.mybir as mybir
import concourse.tile as tile
from concourse.bass_utils import run_bass_kernel_spmd
from concourse.vector_clock import ScopedClock


# When True, the next TileContext exit emits only the drain chain (no
# all-engine barrier / semaphore clears).  Safe only for the final context
# of the program: the SP drain chain waits on every semaphore, so SP ends
# last and NEFF completion still implies all work (incl. DMA) is done.
_SKIP_TAIL_BARRIER = False


def _split_drain_and_barrier(self, tick_clock, wait_clock):
    """Replacement for TileContext._drain_and_barrier that splits the tail
    drain's semaphore waits across multiple single-wait Drain instructions.
    The walrus build in this environment rejects instructions with more than
    a few sync waits ("Too many sync wait commands"), and the stock tail
    drain waits on every semaphore the kernel used."""
    nc = self.nc
    drain_inst = nc.sync.drain()
    wait_clock.add_sem_waits(
        drain_inst.ins, ScopedClock({None: tick_clock.global_clock})
    )
    si = drain_inst.ins.sync_info
    if si is not None and len(si.on_wait) > 1:
        waits = list(si.on_wait)
        updates = list(si.on_update)
        drain_inst.ins.sync_info = mybir.SyncInfo(on_wait=waits[:1], on_update=[])
        for i, w in enumerate(waits[1:]):
            last = i == len(waits) - 2
            d = nc.sync.drain()
            d.ins.sync_info = mybir.SyncInfo(
                on_wait=[w], on_update=updates if last else []
            )

    assert self.sems is not None
    popped = nc._tile_sem_poison_stack.pop()
    assert popped is self._sem_poison
    if _SKIP_TAIL_BARRIER:
        return
    nc.all_engine_barrier()
    nc.clear_and_free_semaphores(list(self.sems.allocated().values()))
    nc.all_engine_barrier()


tile.TileContext._drain_and_barrier = _split_drain_and_barrier

N = 8192            # rows of inputs
D = 512             # feature dim (contraction)
B = 512             # output block size
NB = N // B         # 16 blocks per side
C = 8               # cores
P = 128
KSUB = D // P       # 4 k-subtiles
MSUB = B // P       # 4 m-subtiles per block

# (m_block_local, col_block_local) per output slot, ordered by column so
# compute can start as soon as the first xt column-block lands in SBUF.
# m_block_local 0 -> local rows [0, 512); 1 -> local rows [4096, 4608).
SLOTS = sorted(
    [(0, j) for j in range(9)] + [(1, j) for j in range(8, 16)],
    key=lambda t: (t[1], t[0]),
)
NSLOT = len(SLOTS)  # 17

# store-batch boundaries: after slot s, store slots [STORE_AFTER[s], s].
# At most ~6 store DMAs + ~5 load DMAs so the total semaphore count stays
# under the final Drain instruction's sync-wait limit.
_batch_sizes = [2, 2, 2, 2, 2, 2, 2, 3]
STORE_AFTER = {}
_lo = 0
for _sz in _batch_sizes:
    STORE_AFTER[_lo + _sz - 1] = _lo
    _lo += _sz
assert _lo == NSLOT

# xt load groups: (start, size) per DMA.  Finer first groups let the PE
# start after ~0.5 MB instead of 2.25 MB; 7 load DMAs + st = 8 SWDGE queues.
XT_LOAD_GROUPS = [(0, 1), (1, 1), (2, 2), (4, 2), (6, 2), (8, 4), (12, 4)]
_XT_GROUP_OF = {}
for _gi, (_j0, _sz) in enumerate(XT_LOAD_GROUPS):
    for _j in range(_j0, _j0 + _sz):
        _XT_GROUP_OF[_j] = _gi
assert len(_XT_GROUP_OF) == NB


def _build_nc(repeats: int = 1, **body_kwargs):
    """Build the SPMD program.  repeats>1 runs the whole body that many
    times as sequential TileContexts (used only for timing: the delta
    between repeats isolates device time from dispatch overhead)."""
    nc = bass.Bass()
    # host-pretiled layouts: xt[j, p, o, v] = XT col-block j; st[p, o, m]
    xt = nc.declare_dram_parameter(
        "xt", [NB, P, KSUB, B], mybir.dt.bfloat16, isOutput=False
    )
    st = nc.declare_dram_parameter(
        "st", [P, KSUB, 2 * B], mybir.dt.bfloat16, isOutput=False
    )
    # partition-major output: out[p, s*4+mi, v] = block s row (mi*128+p) col v.
    # Makes every store DMA a contiguous per-partition blit of the staging
    # tile; the host untangles the layout during assembly.
    out = nc.declare_dram_parameter(
        "out", [P, NSLOT * MSUB, B], mybir.dt.float16, isOutput=True
    )
    global _SKIP_TAIL_BARRIER
    for rep in range(repeats):
        _SKIP_TAIL_BARRIER = rep == repeats - 1
        _emit_body(nc, xt, st, out, **body_kwargs)
    _SKIP_TAIL_BARRIER = False
    return nc


def _emit_body(nc, xt, st, out, do_mm=True, do_copy=True, do_store=True):
    with tile.TileContext(nc) as tc:
        with (
            tc.tile_pool(name="xt", bufs=1) as xt_pool,
            tc.tile_pool(name="st", bufs=1) as st_pool,
            tc.tile_pool(name="ob", bufs=1) as out_pool,
            tc.tile_pool(name="ps", bufs=1, space="PSUM") as psum_pool,
        ):
            st_sb = st_pool.tile([P, KSUB, 2 * B], mybir.dt.bfloat16)
            nc.gpsimd.dma_start(st_sb[:], st[:])

            # Fully-resident xt, loaded in a few grouped DMAs (unique dst,
            # no reuse -> no extra sync waits).  j-major so each load lands
            # contiguously per partition.
            xt_sb = xt_pool.tile([P, NB, KSUB, B], mybir.dt.bfloat16)
            for j0, sz in XT_LOAD_GROUPS:
                nc.gpsimd.dma_start(
                    xt_sb[:, j0 : j0 + sz],
                    xt[j0 : j0 + sz].rearrange("j p o v -> p j o v"),
                )

            # statically rotated PSUM banks; unique fp16 staging slot per
            # output tile (no slot reuse -> single-wait copies and stores).
            psum_big = psum_pool.tile([P, 8, B], mybir.dt.float32)
            ob_big = out_pool.tile([P, NSLOT * MSUB, B], mybir.dt.float16)

            g = 0
            seen_grp = set()
            for s, (mb, j) in enumerate(SLOTS):
                if _XT_GROUP_OF[j] not in seen_grp:
                    # Dummy weight load: makes PE observe the xt DMA here
                    # (Ldweights takes one sync wait), so the following
                    # matmuls only carry the PSUM-reuse wait.  HW allows a
                    # single wait per engine instruction.
                    nc.tensor.ldweights(xt_sb[:, j, 0, 0:P])
                    seen_grp.add(_XT_GROUP_OF[j])
                for mi in range(MSUB):
                    ps = psum_big[:, g % 8, :]
                    m0 = mb * B + mi * P
                    if do_mm:
                        for k in range(KSUB):
                            nc.tensor.matmul(
                                ps,
                                st_sb[:, k, m0 : m0 + P],
                                xt_sb[:, j, k, :],
                                start=(k == 0),
                                stop=(k == KSUB - 1),
                            )
                    if do_copy:
                        nc.vector.tensor_copy(ob_big[:, g, :], ps)
                    g += 1
                # Batched stores: at most 8 output DMAs total (one per HWDGE
                # queue) so no DMA ever needs both a data wait and a
                # queue-reuse wait -- instructions only support 1 sync wait.
                if do_store and s in STORE_AFTER:
                    lo = STORE_AFTER[s]
                    nc.sync.dma_start(
                        out[:, lo * MSUB : (s + 1) * MSUB, :],
                        ob_big[:, lo * MSUB : (s + 1) * MSUB, :],
                    )


def _make_in_maps(inputs: np.ndarray, relation: np.ndarray):
    scaled = (inputs * relation[None, :]).astype(np.float32)
    xb = inputs.astype(ml_dtypes.bfloat16)
    sb = scaled.astype(ml_dtypes.bfloat16)
    in_maps = []
    for c in range(C):
        # local row l corresponds to global row (c*B + l) % N
        xr = np.roll(xb, -c * B, axis=0)          # [8192, 512]
        # xt[j, p, o, v] = xr[j*B + v, o*P + p]
        xt_c = np.ascontiguousarray(
            xr.reshape(NB, B, KSUB, P).transpose(0, 3, 2, 1)
        )
        sr = np.roll(sb, -c * B, axis=0)
        s_rows = np.concatenate([sr[:B], sr[8 * B : 9 * B]], axis=0)  # [1024, 512]
        # st[p, o, m] = s_rows[m, o*P + p]
        st_c = np.ascontiguousarray(
            s_rows.reshape(2 * B, KSUB, P).transpose(2, 1, 0)
        )
        in_maps.append({"xt": xt_c, "st": st_c})
    return in_maps


def _assemble(outs: list) -> np.ndarray:
    rec = np.empty((N, N), dtype=np.float32)
    for c in range(C):
        # [128, 68, 512] partition-major -> [17, 512, 512] blocks
        blocks = np.ascontiguousarray(
            np.asarray(outs[c], dtype=np.float32)
            .reshape(P, NSLOT, MSUB, B)
            .transpose(1, 2, 0, 3)
            .reshape(NSLOT, B, B)
        )
        for s, (mb, j) in enumerate(SLOTS):
            r = (c + mb * 8) % NB
            q = (c + j) % NB
            blk = blocks[s]
            rec[r * B : (r + 1) * B, q * B : (q + 1) * B] = blk
            if q != r:
                rec[q * B : (q + 1) * B, r * B : (r + 1) * B] = blk.T
    return rec


def kernel(inputs: np.ndarray, relation: np.ndarray) -> np.ndarray:
    nc = _build_nc()
    res = run_bass_kernel_spmd(nc, _make_in_maps(inputs, relation), list(range(C)))
    return _assemble([r["out"] for r in res.results])
